# revision 1
# baseline (speedup 1.0000x reference)
import sys
sys.path.insert(0, '/opt/trn_rl_repo')
import numpy as np

P = 128
NCORES = 8
F = 128


def _prep(x, edge_index, n_pad, nblk_per_core):
    """Host-side graph prep: gcn norm, dest-sorted per-core metadata.

    Real edges are packed into columns of 128 per dest block; block b owns
    K_b columns where K_b = max over cores of ceil(edges/128) so the SPMD
    instruction stream is identical on every core. Self-loop terms are not
    gathered; they are applied on-chip via a dis^2-scaled DIAG matmul.
    """
    N = x.shape[0]
    loop_full = np.concatenate([edge_index[1].astype(np.int64), np.arange(N, dtype=np.int64)])
    deg = np.bincount(loop_full, minlength=N).astype(np.float32)
    dis = np.where(deg > 0, 1.0 / np.sqrt(deg), 0.0).astype(np.float32)
    # self-loops are aggregated on-chip via a DIAG matmul; gather real edges only
    row = edge_index[0].astype(np.int64)
    col = edge_index[1].astype(np.int64)
    norm = (dis[row] * dis[col]).astype(np.float32)

    slice_rows = nblk_per_core * P
    nblk_tot = NCORES * nblk_per_core

    order = np.argsort(col, kind='stable')
    row_s = row[order].astype(np.int32)
    col_s = col[order].astype(np.int32)
    nrm_s = norm[order]

    blk_of_edge = col_s // P
    cnt = np.bincount(blk_of_edge, minlength=nblk_tot)
    blk_starts = np.zeros(nblk_tot + 1, np.int64)
    np.cumsum(cnt, out=blk_starts[1:])

    K_b = (-(-cnt // P)).reshape(NCORES, nblk_per_core).max(axis=0)  # [nblk]
    col_start = np.zeros(nblk_per_core + 1, np.int64)
    np.cumsum(K_b, out=col_start[1:])
    ncols = int(col_start[-1])

    metas = []
    for c in range(NCORES):
        midx = np.zeros((P, ncols), np.int32)
        mdlc = np.zeros((P, ncols), np.float32)
        mnrm = np.zeros((P, ncols), np.float32)
        for j in range(nblk_per_core):
            g = c * nblk_per_core + j
            s, e = blk_starts[g], blk_starts[g + 1]
            n = e - s
            kb = int(K_b[j])
            npad = kb * P
            src_p = np.zeros(npad, np.int32)
            dlc_p = np.zeros(npad, np.float32)
            nrm_p = np.zeros(npad, np.float32)
            src_p[:n] = row_s[s:e]
            dlc_p[:n] = (col_s[s:e] - g * P).astype(np.float32)
            nrm_p[:n] = nrm_s[s:e]
            c0 = int(col_start[j])
            # edge t of block -> chunk t//P, partition t%P ; column c0 + chunk
            midx[:, c0:c0 + kb] = src_p.reshape(kb, P).T
            mdlc[:, c0:c0 + kb] = dlc_p.reshape(kb, P).T
            mnrm[:, c0:c0 + kb] = nrm_p.reshape(kb, P).T
        metas.append([midx, mdlc, mnrm])

    layout = {"ncols": ncols, "K_b": K_b, "col_start": col_start}

    disq_pad = np.zeros(n_pad, np.float32)
    disq_pad[:N] = dis * dis
    # disq[d, b] = dis^2 of node (b*128+d) of this core's slice

    x_pad = np.zeros((n_pad, x.shape[1]), np.float32)
    x_pad[:N] = x
    x_f16 = x_pad.astype(np.float16)
    x_slices = [x_f16[c * slice_rows:(c + 1) * slice_rows] for c in range(NCORES)]
    for c in range(NCORES):
        dq = disq_pad[c * slice_rows:(c + 1) * slice_rows].reshape(nblk_per_core, P).T
        metas[c].append(np.ascontiguousarray(dq))  # [128, nblk]
        # prop-0 messages pre-gathered on host: xmsg[p, c*F:(c+1)*F] = x16[midx[p,c]]
        midx = metas[c][0]
        xmsg = x_f16[midx]                          # [128, ncols, F]
        metas[c].append(np.ascontiguousarray(xmsg.reshape(P, ncols * x.shape[1])))
    return metas, x_slices, layout


def _build_bass(n_pad, nblk, layout, weights, biases, Wp, bp):
    """SPMD Bass program (identical on all cores; per-core data via inputs).

    Per propagation i=0..3 over v_i (v0=x, v1=h1, v2=h2, v3=h3), fp16:
      per dest block: per-column indirect gather of source rows, one-hot
      matmul aggregation into gt[f,d] (f32 PSUM), then
        h_{i+1}[d,o] = relu(gt^T W_i + b_i) -> hsl -> funnel -> AllGather (i<3)
        y[d,2]      += gt^T Wp_{i-1}         (i>=1)
      Softmax(2) = sigmoid(y0-y1 + (bp0-bp1)) epilogue.
    """
    from concourse import bass, bacc, mybir
    import concourse.tile as tile

    slice_rows = nblk * P
    ncols = layout["ncols"]
    K_b = layout["K_b"]
    col_start = layout["col_start"]

    nc = bacc.Bacc(num_devices=NCORES, num_swdge_queues=4)

    f16 = mybir.dt.float16
    f32 = mybir.dt.float32
    x_in = nc.declare_dram_parameter("x_in", [slice_rows, F], f16, isOutput=False)
    midx_in = nc.declare_dram_parameter("midx", [P, ncols], mybir.dt.int32, isOutput=False)
    mdlc_in = nc.declare_dram_parameter("mdlc", [P, ncols], f32, isOutput=False)
    mnrm_in = nc.declare_dram_parameter("mnrm", [P, ncols], f32, isOutput=False)
    disq_in = nc.declare_dram_parameter("disq", [P, nblk], f32, isOutput=False)
    xmsg_in = nc.declare_dram_parameter("xmsg", [P, ncols * F], f16, isOutput=False)
    y_out = nc.declare_dram_parameter("y_out", [slice_rows, 2], f32, isOutput=True)

    h_slice = [nc.dram_tensor(f"h_slice{i}", [slice_rows, F], f16) for i in range(3)]
    v_full = [None] + [nc.dram_tensor(f"v_full{i}", [n_pad, F], f16, addr_space="Shared")
                       for i in range(1, 4)]

    iota_np = np.broadcast_to(np.arange(P, dtype=np.float32), (P, P)).astype(np.float16).copy()
    iota_d = nc.inline_tensor(iota_np, name="iota_c")
    iotac_d = nc.inline_tensor(np.arange(P, dtype=np.float32).reshape(P, 1), name="iotacol")
    W_d = [nc.inline_tensor(np.ascontiguousarray(w).astype(np.float16), name=f"W{i}")
           for i, w in enumerate(weights)]
    B_d = [nc.inline_tensor(np.broadcast_to(b.astype(np.float32), (P, F)).copy(), name=f"B{i}")
           for i, b in enumerate(biases)]
    Wp_d = [nc.inline_tensor(np.ascontiguousarray(Wp[i * F:(i + 1) * F, :]).astype(np.float16),
                             name=f"Wp{i}") for i in range(3)]
    bp_diff = float(bp[0]) - float(bp[1])

    AF = mybir.ActivationFunctionType
    ALU = mybir.AluOpType
    rg = [list(range(NCORES))]

    with tile.TileContext(nc) as tc:
        with (
            tc.tile_pool(name="const", bufs=1) as cpool,
            tc.tile_pool(name="msg", bufs=24) as mpool,
            tc.tile_pool(name="msgx", bufs=6) as mxpool,
            tc.tile_pool(name="s", bufs=8) as spool,
            tc.tile_pool(name="work", bufs=6) as wpool,
            tc.tile_pool(name="epi", bufs=1) as epool,
            tc.tile_pool(name="psA", bufs=4, space="PSUM") as psA,
            tc.tile_pool(name="psB", bufs=2, space="PSUM") as psB,
            tc.tile_pool(name="psC", bufs=2, space="PSUM") as psC,
        ):
            iota_t = cpool.tile([P, P], f16)
            nc.sync.dma_start(out=iota_t[:], in_=iota_d[:, :])
            iotac_t = cpool.tile([P, 1], f32)
            nc.sync.dma_start(out=iotac_t[:], in_=iotac_d[:, :])
            W_t, B_t, Wp_t = [], [], []
            for i in range(3):
                wt = cpool.tile([P, F], f16, tag=f"w{i}")
                nc.sync.dma_start(out=wt[:], in_=W_d[i][:, :])
                W_t.append(wt)
                bt = cpool.tile([P, F], f32, tag=f"b{i}")
                nc.sync.dma_start(out=bt[:], in_=B_d[i][:, :])
                B_t.append(bt)
                wpt = cpool.tile([P, 2], f16, tag=f"wp{i}")
                nc.sync.dma_start(out=wpt[:], in_=Wp_d[i][:, :])
                Wp_t.append(wpt)

            midx_t = cpool.tile([P, ncols], mybir.dt.int32)
            mdlc_t = cpool.tile([P, ncols], f32)
            mnrm_t = cpool.tile([P, ncols], f32)
            disq_t = cpool.tile([P, nblk], f32)
            nc.sync.dma_start(out=midx_t[:], in_=midx_in[:, :])
            nc.sync.dma_start(out=mdlc_t[:], in_=mdlc_in[:, :])
            nc.sync.dma_start(out=mnrm_t[:], in_=mnrm_in[:, :])
            nc.sync.dma_start(out=disq_t[:], in_=disq_in[:, :])

            ysb = cpool.tile([P, 2 * nblk], f32)
            nc.vector.memset(ysb[:], 0.0)
            hsl = cpool.tile([P, slice_rows], f16)
            # node-major x slice for prop-0 self-loop terms
            xsl = cpool.tile([P, slice_rows], f16)
            nc.sync.dma_start(
                out=xsl[:].rearrange("d (b o) -> d b o", o=F),
                in_=x_in.rearrange("(b d) o -> d b o", d=P))

            kbmax = int(max(int(K_b[b]) for b in range(nblk)))
            # ---- 4 propagations ----
            for i in range(4):
                src = v_full[i]
                for b in range(nblk):
                    kb = int(K_b[b])
                    c0 = int(col_start[b])
                    gt = psA.tile([P, P], f32, tag="gt", space="PSUM")
                    if i == 0 and b % 2 == 0:
                        # prop-0 messages are host-pregathered; adjacent blocks'
                        # columns are contiguous, so load them pairwise
                        npair = kb + (int(K_b[b + 1]) if b + 1 < nblk else 0)
                        msgpair = mxpool.tile([P, 2 * kbmax * F], f16, tag="msgx")
                        nc.sync.dma_start(out=msgpair[:, :npair * F],
                                          in_=xmsg_in[:, c0 * F:(c0 + npair) * F])
                        pair_base = c0
                    # self-loop term: gt[f,d] += h_i[d,f] * dis^2[d] via one-hot diag
                    DIAG = spool.tile([P, P], f16, tag="S")
                    nc.vector.tensor_scalar(
                        out=DIAG[:], in0=iota_t[:],
                        scalar1=iotac_t[:, 0:1], op0=ALU.is_equal,
                        scalar2=disq_t[:, b:b + 1], op1=ALU.mult,
                    )
                    selfsrc = xsl if i == 0 else hsl
                    nc.tensor.matmul(out=gt[:], lhsT=selfsrc[:, b * P:(b + 1) * P],
                                     rhs=DIAG[:], start=True, stop=(kb == 0))
                    for k in range(kb):
                        col = c0 + k
                        if i == 0:
                            po = col - pair_base
                            lhs = msgpair[:, po * F:(po + 1) * F]
                        else:
                            msg = mpool.tile([P, F], f16, tag="msg")
                            gins = nc.gpsimd.indirect_dma_start(
                                out=msg[:], out_offset=None,
                                in_=src[:],
                                in_offset=bass.IndirectOffsetOnAxis(
                                    ap=midx_t[:, col:col + 1], axis=0),
                            )
                            qn = col % 4
                            gins.ins.queue = f"qPoolDynamic{qn or ''}"
                            lhs = msg[:]
                        S = spool.tile([P, P], f16, tag="S")
                        nc.vector.tensor_scalar(
                            out=S[:], in0=iota_t[:],
                            scalar1=mdlc_t[:, col:col + 1], op0=ALU.is_equal,
                            scalar2=mnrm_t[:, col:col + 1], op1=ALU.mult,
                        )
                        nc.tensor.matmul(out=gt[:], lhsT=lhs, rhs=S[:],
                                         start=False, stop=(k == kb - 1))
                    gts = wpool.tile([P, P], f16, tag="gts")
                    nc.scalar.copy(out=gts[:], in_=gt[:])
                    if i < 3:
                        hp = psB.tile([P, P], f32, tag="hx", space="PSUM")
                        nc.tensor.matmul(out=hp[:], lhsT=gts[:], rhs=W_t[i][:],
                                         start=True, stop=True)
                        hb = hsl[:, b * P:(b + 1) * P]
                        nc.vector.tensor_tensor(out=hb, in0=hp[:], in1=B_t[i][:], op=ALU.add)
                        nc.vector.tensor_scalar_max(out=hb, in0=hb, scalar1=0.0)
                    if i >= 1:
                        yp = psC.tile([P, 2], f32, tag="yp", space="PSUM")
                        nc.tensor.matmul(out=yp[:], lhsT=gts[:], rhs=Wp_t[i - 1][:],
                                         start=True, stop=True)
                        ys = ysb[:, 2 * b:2 * b + 2]
                        nc.vector.tensor_tensor(out=ys, in0=ys, in1=yp[:], op=ALU.add)
                if i < 3:
                    nc.sync.dma_start(
                        out=h_slice[i].rearrange("(b d) o -> d b o", d=P),
                        in_=hsl[:].rearrange("d (b o) -> d b o", o=F))
                    nc.gpsimd.collective_compute(
                        "AllGather", ALU.bypass, replica_groups=rg,
                        ins=[h_slice[i][:].opt()], outs=[v_full[i + 1][:].opt()],
                    )

            # ---- epilogue: softmax(2) = sigmoid(y0-y1+bp0-bp1); y funnel ----
            yv = ysb[:].rearrange("d (b o) -> d b o", o=2)
            dif = epool.tile([P, nblk], f32, tag="dif")
            nc.vector.tensor_tensor(out=dif[:], in0=yv[:, :, 0:1].opt(),
                                    in1=yv[:, :, 1:2].opt(), op=ALU.subtract)
            nc.vector.tensor_scalar_add(out=dif[:], in0=dif[:], scalar1=bp_diff)
            youtsb = epool.tile([P, 2 * nblk], f32, tag="yo")
            yov = youtsb[:].rearrange("d (b o) -> d b o", o=2)
            sig = epool.tile([P, nblk], f32, tag="sig")
            nc.scalar.activation(out=sig[:], in_=dif[:], func=AF.Sigmoid)
            nc.vector.tensor_copy(out=yov[:, :, 0:1].opt(), in_=sig[:])
            nc.vector.tensor_scalar(out=yov[:, :, 1:2].opt(), in0=sig[:],
                                    scalar1=-1.0, op0=ALU.mult,
                                    scalar2=1.0, op1=ALU.add)
            nc.sync.dma_start(
                out=y_out.rearrange("(b d) o -> d b o", d=P),
                in_=youtsb[:].rearrange("d (b o) -> d b o", o=2))

    nc.compile()
    return nc


LAST_RESULTS = None
LAST_NC = None
LAST_IN_MAPS = None


def kernel(x, edge_index, W0, b0, W1, b1, W2, b2, Wp, bp):
    global LAST_RESULTS, LAST_NC, LAST_IN_MAPS
    import os
    from concourse.bass_utils import run_bass_kernel_spmd

    x = np.asarray(x, dtype=np.float32)
    edge_index = np.asarray(edge_index)
    N = x.shape[0]
    nblk_per_core = int(np.ceil(N / (NCORES * P)))
    n_pad = NCORES * nblk_per_core * P
    slice_rows = nblk_per_core * P

    # degree-balanced relabeling: deal nodes (sorted by in-degree) round-robin
    # across all global blocks so per-block edge counts equalize, shrinking the
    # max-over-cores column padding K_b.
    nblk_tot = NCORES * nblk_per_core
    indeg = np.bincount(edge_index[1].astype(np.int64), minlength=N)
    order_deg = np.argsort(-indeg, kind='stable')
    perm = np.empty(N, np.int64)              # old id -> new slot
    nfull = (N // nblk_tot) * nblk_tot
    nstripe = nfull // nblk_tot
    fwd = np.arange(nblk_tot)
    blk_seq = np.concatenate([fwd if s % 2 == 0 else fwd[::-1] for s in range(nstripe)])
    slot_in_blk = np.repeat(np.arange(nstripe), nblk_tot)
    newid_full = blk_seq * P + slot_in_blk
    rem = N - nfull
    # leftovers go to the first `rem` blocks at the next slot
    newid_rem = np.arange(rem) * P + (nfull // nblk_tot)
    newid = np.concatenate([newid_full, newid_rem])
    perm[order_deg] = newid
    x_rel = np.zeros((n_pad, x.shape[1]), np.float32)
    x_rel[perm] = x
    edge_rel = perm[edge_index.astype(np.int64)]

    metas, x_slices, layout = _prep(x_rel, edge_rel, n_pad, nblk_per_core)

    nc = _build_bass(
        n_pad, nblk_per_core, layout,
        [np.asarray(W0), np.asarray(W1), np.asarray(W2)],
        [np.asarray(b0), np.asarray(b1), np.asarray(b2)],
        np.asarray(Wp), np.asarray(bp),
    )

    in_maps = []
    for c in range(NCORES):
        midx, mdlc, mnrm, disq, xmsg = metas[c]
        in_maps.append({
            "x_in": np.ascontiguousarray(x_slices[c]),
            "midx": midx, "mdlc": mdlc, "mnrm": mnrm, "disq": disq,
            "xmsg": xmsg,
        })

    trace = bool(os.environ.get("KERNEL_TRACE"))
    res = run_bass_kernel_spmd(nc, in_maps, list(range(NCORES)), trace=trace)
    LAST_RESULTS = res
    LAST_NC = nc
    LAST_IN_MAPS = in_maps

    out = np.zeros((n_pad, 2), np.float32)
    for c in range(NCORES):
        out[c * slice_rows:(c + 1) * slice_rows] = res.results[c]["y_out"]
    return out[perm]



# revision 7
# speedup vs baseline: 1.0496x; 1.0496x over previous
import sys
sys.path.insert(0, '/opt/trn_rl_repo')
import numpy as np

P = 128
NCORES = 8
F = 128
WSIZE = 32768          # dma_gather int16 index range
WSTARTS = (0, 22528, 45056, 67584)   # overlapping source windows
NW = 4
QUOTA = 512            # per-(block,window) edge quota for balance
GBLK = 8               # dest blocks per gather group
# Max columns (x128 idx) per dma_gather call: the SWDGE packs 16 idx per
# descriptor and a single packed gather must stay under the HW packet/ring
# entry budget (2048-idx gathers wedge the device; 1024-idx ones work).
GMAX_COLS = 8


def _prep(x, edge_index, n_pad, nblk_per_core):
    """Host-side graph prep: gcn norm, per-core dest-block/window metadata.

    Norm factorization: norm_e = dis[src]*dis[dst]. The dis[src] factor is
    folded into the stored node features (x and every h are stored scaled by
    dis), and the dis[dst] factor is applied after the aggregation matmuls.
    The scatter matrices are therefore pure 0/1 one-hots, built per dest
    block in a single broadcast is_equal, and the self-loop term reduces to
    an identity matmul of the (already dis-scaled) features.

    Edges are bucketed per (dest block, source window); each bucket is padded
    to K*128 slots where K = max over cores of ceil(count/128), so the SPMD
    instruction stream is identical on every core. Gathers are bulk
    dma_gather ops; indices are int16 window-relative source ids in the
    wrapped [16, n/16] layout replicated across the 8 Q7 cores.
    """
    N = x.shape[0]  # == n_pad (x is padded)
    loop_full = np.concatenate([edge_index[1].astype(np.int64), np.arange(N, dtype=np.int64)])
    deg = np.bincount(loop_full, minlength=N).astype(np.float32)
    dis = np.where(deg > 0, 1.0 / np.sqrt(deg), 0.0).astype(np.float32)
    row = edge_index[0].astype(np.int64)
    col = edge_index[1].astype(np.int64)

    nblk_tot = NCORES * nblk_per_core
    order = np.argsort(col, kind='stable')
    row_s = row[order]
    col_s = col[order]
    blk_of = col_s // P
    cnt_blk = np.bincount(blk_of, minlength=nblk_tot)
    starts = np.zeros(nblk_tot + 1, np.int64)
    np.cumsum(cnt_blk, out=starts[1:])

    # per (core, block): sort by src, assign to windows with quota balancing
    seg = {}
    cnts = np.zeros((NCORES, nblk_per_core, NW), np.int64)
    for c in range(NCORES):
        for j in range(nblk_per_core):
            g = c * nblk_per_core + j
            s, e = starts[g], starts[g + 1]
            so = np.argsort(row_s[s:e], kind='stable')
            srcs = row_s[s:e][so]
            dsts = col_s[s:e][so]
            ptr = 0
            for w in range(NW):
                if w < NW - 1:
                    must = int(np.searchsorted(srcs, WSTARTS[w + 1]))
                    elig = int(np.searchsorted(srcs, WSTARTS[w] + WSIZE))
                    take = max(must, min(elig, ptr + QUOTA))
                else:
                    take = len(srcs)
                cnts[c, j, w] = take - ptr
                seg[(c, j, w)] = (
                    (srcs[ptr:take] - WSTARTS[w]).astype(np.int16),
                    (dsts[ptr:take] - g * P).astype(np.float16),
                )
                ptr = take

    Khat = (-(-cnts // P)).max(axis=0)  # [nblk, NW]
    kflat = Khat.reshape(-1)
    col_start = np.zeros(nblk_per_core * NW + 1, np.int64)
    np.cumsum(kflat, out=col_start[1:])
    ncols = int(col_start[-1])

    groups = [list(range(a, min(nblk_per_core, a + GBLK)))
              for a in range(0, nblk_per_core, GBLK)]
    gw_cols = [[int(sum(Khat[j][w] for j in grp)) for w in range(NW)]
               for grp in groups]
    gw_off8 = []
    acc = 0
    for gi in range(len(groups)):
        offs = []
        for w in range(NW):
            offs.append(acc)
            acc += gw_cols[gi][w] * 8  # num_idxs/16 = cols*8 int16 per partition
        gw_off8.append(offs)
    midx_width = acc

    metas = []
    for c in range(NCORES):
        mdlc = np.full((P, ncols), -1.0, np.float16)
        midx = np.zeros((P, midx_width), np.int16)
        for gi, grp in enumerate(groups):
            for w in range(NW):
                ncgw = gw_cols[gi][w]
                if ncgw == 0:
                    continue
                arr = np.zeros(ncgw * P, np.int16)
                loc = 0
                for j in grp:
                    K = int(Khat[j][w])
                    if K == 0:
                        continue
                    src16, dlc = seg[(c, j, w)]
                    n = len(src16)
                    npad_ = K * P
                    a = np.zeros(npad_, np.int16)
                    a[:n] = src16
                    d = np.full(npad_, -1.0, np.float16)
                    d[:n] = dlc
                    c0 = int(col_start[j * NW + w])
                    # slot t of bucket -> column c0 + t//128, partition t%128
                    mdlc[:, c0:c0 + K] = d.reshape(K, P).T
                    arr[loc * P:(loc + K) * P] = a
                    loc += K
                wrapped = arr.reshape(-1, 16).T      # [16, n/16]; slot i at [i%16, i//16]
                rep = np.tile(wrapped, (8, 1))       # replicate for 8 Q7 cores
                o = gw_off8[gi][w]
                midx[:, o:o + ncgw * 8] = rep
        metas.append([midx, mdlc])

    layout = dict(ncols=ncols, Khat=Khat, col_start=col_start, groups=groups,
                  gw_cols=gw_cols, gw_off8=gw_off8, midx_width=midx_width)

    slice_rows = nblk_per_core * P
    # store features pre-scaled by dis (the dis[src] half of the edge norm)
    xs = (x * dis[:, None]).astype(np.float16)
    x_slices = [xs[c * slice_rows:(c + 1) * slice_rows] for c in range(NCORES)]
    for c in range(NCORES):
        dc = dis[c * slice_rows:(c + 1) * slice_rows].reshape(nblk_per_core, P).T
        metas[c].append(np.ascontiguousarray(dc.astype(np.float32)))  # [128, nblk]
    return metas, x_slices, xs, layout


def _build_bass(n_pad, nblk, layout, weights, biases, Wp, bp):
    """SPMD Bass program (identical on all cores; per-core data via inputs).

    Per propagation i=0..3 over scaled features v_i = h_i*dis (v0=x*dis):
      per (block-group, window): bulk dma_gather of source rows (<=1024 idx
      per call); per dest block: identity self-loop matmul plus one-hot
      scatter matmuls into gt[f,d] (f32 PSUM) using a block-wide broadcast
      is_equal one-hot, then (with the dis[dst] factor applied on the scalar
      engine):
        v_{i+1}[d,o] = relu(dis_d*(gt^T W_i) + b_i)*dis_d     (i<3)
        y'[d,2]     += gt^T Wp_{i-1}                          (i>=1)
      Epilogue: softmax(2) = sigmoid(dis_d*(y0'-y1') + bp0-bp1).
    """
    from concourse import bass, bacc, mybir
    import concourse.tile as tile

    slice_rows = nblk * P
    ncols = layout["ncols"]
    Khat = layout["Khat"]
    col_start = layout["col_start"]
    groups = layout["groups"]
    gw_cols = layout["gw_cols"]
    gw_off8 = layout["gw_off8"]
    midx_width = layout["midx_width"]

    nc = bacc.Bacc(num_devices=NCORES, num_swdge_queues=4)

    f16 = mybir.dt.float16
    f32 = mybir.dt.float32
    i16 = mybir.dt.int16
    x_in = nc.declare_dram_parameter("x_in", [slice_rows, F], f16, isOutput=False)
    x_full = nc.declare_dram_parameter("x_full", [n_pad, F], f16, isOutput=False)
    midx_in = nc.declare_dram_parameter("midx", [P, midx_width], i16, isOutput=False)
    mdlc_in = nc.declare_dram_parameter("mdlc", [P, ncols], f16, isOutput=False)
    discol_in = nc.declare_dram_parameter("discol", [P, nblk], f32, isOutput=False)
    y_out = nc.declare_dram_parameter("y_out", [slice_rows, 2], f32, isOutput=True)

    h_slice = [nc.dram_tensor(f"h_slice{i}", [slice_rows, F], f16) for i in range(3)]
    v_full = [x_full] + [nc.dram_tensor(f"v_full{i}", [n_pad, F], f16, addr_space="Shared")
                         for i in range(1, 4)]

    iota_np = np.broadcast_to(np.arange(P, dtype=np.float32), (P, P)).astype(np.float16).copy()
    iota_d = nc.inline_tensor(iota_np, name="iota_c")
    ident_d = nc.inline_tensor(np.eye(P, dtype=np.float16), name="ident")
    W_d = [nc.inline_tensor(np.ascontiguousarray(w).astype(np.float16), name=f"W{i}")
           for i, w in enumerate(weights)]
    B_d = [nc.inline_tensor(np.broadcast_to(b.astype(np.float32), (P, F)).copy(), name=f"B{i}")
           for i, b in enumerate(biases)]
    Wp_d = [nc.inline_tensor(np.ascontiguousarray(Wp[i * F:(i + 1) * F, :]).astype(np.float16),
                             name=f"Wp{i}") for i in range(3)]
    bp_diff = float(bp[0]) - float(bp[1])

    AF = mybir.ActivationFunctionType
    ALU = mybir.AluOpType
    rg = [list(range(NCORES))]

    with tile.TileContext(nc) as tc:
        with (
            tc.tile_pool(name="const", bufs=1) as cpool,
            tc.tile_pool(name="msg", bufs=2) as mpool,
            tc.tile_pool(name="s", bufs=6) as spool,
            tc.tile_pool(name="work", bufs=6) as wpool,
            tc.tile_pool(name="epi", bufs=1) as epool,
            tc.tile_pool(name="psA", bufs=4, space="PSUM") as psA,
            tc.tile_pool(name="psB", bufs=2, space="PSUM") as psB,
            tc.tile_pool(name="psC", bufs=2, space="PSUM") as psC,
        ):
            iota_t = cpool.tile([P, P], f16)
            nc.sync.dma_start(out=iota_t[:], in_=iota_d[:, :])
            ident_t = cpool.tile([P, P], f16)
            nc.sync.dma_start(out=ident_t[:], in_=ident_d[:, :])
            W_t, B_t, Wp_t = [], [], []
            for i in range(3):
                wt = cpool.tile([P, F], f16, tag=f"w{i}")
                nc.sync.dma_start(out=wt[:], in_=W_d[i][:, :])
                W_t.append(wt)
                bt = cpool.tile([P, F], f32, tag=f"b{i}")
                nc.sync.dma_start(out=bt[:], in_=B_d[i][:, :])
                B_t.append(bt)
                wpt = cpool.tile([P, 2], f16, tag=f"wp{i}")
                nc.sync.dma_start(out=wpt[:], in_=Wp_d[i][:, :])
                Wp_t.append(wpt)

            midx_t = cpool.tile([P, midx_width], i16)
            mdlc_t = cpool.tile([P, ncols], f16)
            discol_t = cpool.tile([P, nblk], f32)
            nc.sync.dma_start(out=midx_t[:], in_=midx_in[:, :])
            nc.sync.dma_start(out=mdlc_t[:], in_=mdlc_in[:, :])
            nc.sync.dma_start(out=discol_t[:], in_=discol_in[:, :])

            ysb = cpool.tile([P, 2 * nblk], f32)
            nc.vector.memset(ysb[:], 0.0)
            hsl = cpool.tile([P, slice_rows], f16)
            # node-major x*dis slice for prop-0 self-loop terms
            xsl = cpool.tile([P, slice_rows], f16)
            nc.sync.dma_start(
                out=xsl[:].rearrange("d (b o) -> d b o", o=F),
                in_=x_in.rearrange("(b d) o -> d b o", d=P))

            # ---- 4 propagations ----
            # qn is a single global counter: Tile round-robins Pool-DMA insts
            # over 8 DMASW sem lanes, and each lane is locked to one SWDGE
            # queue; queue = counter % 4 keeps lane % 4 == queue always.
            qn = 0
            for i in range(4):
                src = v_full[i]
                for gi, grp in enumerate(groups):
                    # bulk-gather this group's source rows
                    mt = {}
                    for w in range(NW):
                        ncgw = gw_cols[gi][w]
                        if ncgw == 0:
                            continue
                        t = mpool.tile([P, ncgw * F], f16, tag=f"msg{w}")
                        # split into sub-gathers of <= GMAX_COLS columns to
                        # stay under the per-instruction SWDGE budget
                        for c0 in range(0, ncgw, GMAX_COLS):
                            c1 = min(c0 + GMAX_COLS, ncgw)
                            nidx = (c1 - c0) * P
                            o8 = gw_off8[gi][w] + c0 * 8
                            nc.gpsimd.dma_gather(
                                out_ap=t[:, c0 * F:c1 * F].rearrange(
                                    "p (c e) -> p c e", e=F),
                                in_ap=src[WSTARTS[w]:min(WSTARTS[w] + WSIZE, n_pad), :],
                                idxs_ap=midx_t[:, o8:o8 + (c1 - c0) * 8],
                                num_idxs=nidx,
                                num_idxs_reg=nidx,
                                elem_size=F,
                                queue_num=qn % 4,
                            )
                            qn += 1
                        mt[w] = t
                    for j in grp:
                        ktot = int(Khat[j].sum())
                        bc0 = int(col_start[j * NW])
                        gt = psA.tile([P, P], f32, tag="gt", space="PSUM")
                        # block-wide 0/1 one-hot: S[p, k, e] = (e == dlc[p, k])
                        if ktot > 0:
                            Sblk = spool.tile([P, ktot * P], f16, tag="S")
                            nc.vector.tensor_tensor(
                                out=Sblk[:].rearrange("p (k e) -> p k e", e=P),
                                in0=iota_t[:].unsqueeze(1).broadcast_to([P, ktot, P]),
                                in1=mdlc_t[:, bc0:bc0 + ktot].unsqueeze(2)
                                    .broadcast_to([P, ktot, P]),
                                op=ALU.is_equal,
                            )
                        # self-loop: stored features are already dis-scaled
                        selfsrc = xsl if i == 0 else hsl
                        nc.tensor.matmul(out=gt[:], lhsT=selfsrc[:, j * P:(j + 1) * P],
                                         rhs=ident_t[:], start=True, stop=(ktot == 0))
                        done = 0
                        for w in range(NW):
                            K = int(Khat[j][w])
                            if K == 0:
                                continue
                            loc = int(sum(Khat[j2][w] for j2 in grp if j2 < j))
                            krel = int(col_start[j * NW + w]) - bc0
                            for k in range(K):
                                lhs = mt[w][:, (loc + k) * F:(loc + k + 1) * F]
                                done += 1
                                nc.tensor.matmul(
                                    out=gt[:], lhsT=lhs,
                                    rhs=Sblk[:, (krel + k) * P:(krel + k + 1) * P],
                                    start=False, stop=(done == ktot))
                        gts = wpool.tile([P, P], f16, tag="gts")
                        nc.scalar.copy(out=gts[:], in_=gt[:])
                        if i < 3:
                            hp = psB.tile([P, P], f32, tag="hx", space="PSUM")
                            nc.tensor.matmul(out=hp[:], lhsT=gts[:], rhs=W_t[i][:],
                                             start=True, stop=True)
                            # true h = relu(dis_d*hp + b); stored scaled by dis_d
                            tmp = wpool.tile([P, P], f32, tag="tmp")
                            nc.scalar.activation(out=tmp[:], in_=hp[:], func=AF.Copy,
                                                 scale=discol_t[:, j:j + 1])
                            nc.vector.tensor_tensor(out=tmp[:], in0=tmp[:],
                                                    in1=B_t[i][:], op=ALU.add)
                            hb = hsl[:, j * P:(j + 1) * P]
                            nc.scalar.activation(out=hb, in_=tmp[:], func=AF.Relu,
                                                 scale=discol_t[:, j:j + 1])
                        if i >= 1:
                            yp = psC.tile([P, 2], f32, tag="yp", space="PSUM")
                            nc.tensor.matmul(out=yp[:], lhsT=gts[:], rhs=Wp_t[i - 1][:],
                                             start=True, stop=True)
                            ys = ysb[:, 2 * j:2 * j + 2]
                            nc.vector.tensor_tensor(out=ys, in0=ys, in1=yp[:], op=ALU.add)
                    # incremental dump of this group's finished h rows
                    if i < 3:
                        b0, b1 = grp[0], grp[-1] + 1
                        nc.sync.dma_start(
                            out=h_slice[i].rearrange("(b d) o -> d b o", d=P)[:, b0:b1, :],
                            in_=hsl[:, b0 * F:b1 * F].rearrange("d (b o) -> d b o", o=F))
                if i < 3:
                    nc.gpsimd.collective_compute(
                        "AllGather", ALU.bypass, replica_groups=rg,
                        ins=[h_slice[i][:].opt()], outs=[v_full[i + 1][:].opt()],
                    )

            # ---- epilogue: softmax(2) = sigmoid(dis*(y0'-y1')+bp0-bp1) ----
            yv = ysb[:].rearrange("d (b o) -> d b o", o=2)
            dif = epool.tile([P, nblk], f32, tag="dif")
            nc.vector.tensor_tensor(out=dif[:], in0=yv[:, :, 0:1].opt(),
                                    in1=yv[:, :, 1:2].opt(), op=ALU.subtract)
            nc.vector.tensor_tensor(out=dif[:], in0=dif[:], in1=discol_t[:],
                                    op=ALU.mult)
            nc.vector.tensor_scalar_add(out=dif[:], in0=dif[:], scalar1=bp_diff)
            youtsb = epool.tile([P, 2 * nblk], f32, tag="yo")
            yov = youtsb[:].rearrange("d (b o) -> d b o", o=2)
            sig = epool.tile([P, nblk], f32, tag="sig")
            nc.scalar.activation(out=sig[:], in_=dif[:], func=AF.Sigmoid)
            nc.vector.tensor_copy(out=yov[:, :, 0:1].opt(), in_=sig[:])
            nc.vector.tensor_scalar(out=yov[:, :, 1:2].opt(), in0=sig[:],
                                    scalar1=-1.0, op0=ALU.mult,
                                    scalar2=1.0, op1=ALU.add)
            nc.sync.dma_start(
                out=y_out.rearrange("(b d) o -> d b o", d=P),
                in_=youtsb[:].rearrange("d (b o) -> d b o", o=2))

    nc.compile()
    return nc


LAST_RESULTS = None
LAST_NC = None
LAST_IN_MAPS = None


def kernel(x, edge_index, W0, b0, W1, b1, W2, b2, Wp, bp):
    global LAST_RESULTS, LAST_NC, LAST_IN_MAPS
    import os
    from concourse.bass_utils import run_bass_kernel_spmd

    x = np.asarray(x, dtype=np.float32)
    edge_index = np.asarray(edge_index)
    N = x.shape[0]
    nblk_per_core = int(np.ceil(N / (NCORES * P)))
    n_pad = NCORES * nblk_per_core * P
    slice_rows = nblk_per_core * P

    # degree-balanced relabeling: deal nodes (sorted by in-degree) round-robin
    # across all global blocks so per-block edge counts equalize, shrinking the
    # max-over-cores column padding.
    nblk_tot = NCORES * nblk_per_core
    indeg = np.bincount(edge_index[1].astype(np.int64), minlength=N)
    order_deg = np.argsort(-indeg, kind='stable')
    perm = np.empty(N, np.int64)              # old id -> new slot
    nfull = (N // nblk_tot) * nblk_tot
    nstripe = nfull // nblk_tot
    fwd = np.arange(nblk_tot)
    blk_seq = np.concatenate([fwd if s % 2 == 0 else fwd[::-1] for s in range(nstripe)])
    slot_in_blk = np.repeat(np.arange(nstripe), nblk_tot)
    newid_full = blk_seq * P + slot_in_blk
    rem = N - nfull
    newid_rem = np.arange(rem) * P + (nfull // nblk_tot)
    newid = np.concatenate([newid_full, newid_rem])
    perm[order_deg] = newid
    x_rel = np.zeros((n_pad, x.shape[1]), np.float32)
    x_rel[perm] = x
    edge_rel = perm[edge_index.astype(np.int64)]

    metas, x_slices, x_sc16, layout = _prep(x_rel, edge_rel, n_pad, nblk_per_core)

    nc = _build_bass(
        n_pad, nblk_per_core, layout,
        [np.asarray(W0), np.asarray(W1), np.asarray(W2)],
        [np.asarray(b0), np.asarray(b1), np.asarray(b2)],
        np.asarray(Wp), np.asarray(bp),
    )

    in_maps = []
    for c in range(NCORES):
        midx, mdlc, discol = metas[c]
        in_maps.append({
            "x_in": np.ascontiguousarray(x_slices[c]),
            "x_full": x_sc16,
            "midx": midx, "mdlc": mdlc, "discol": discol,
        })

    trace = bool(os.environ.get("KERNEL_TRACE"))
    res = run_bass_kernel_spmd(nc, in_maps, list(range(NCORES)), trace=trace)
    LAST_RESULTS = res
    LAST_NC = nc
    LAST_IN_MAPS = in_maps

    out = np.zeros((n_pad, 2), np.float32)
    for c in range(NCORES):
        out[c * slice_rows:(c + 1) * slice_rows] = res.results[c]["y_out"]
    return out[perm]


# revision 11
# speedup vs baseline: 1.0543x; 1.0045x over previous
import sys
sys.path.insert(0, '/opt/trn_rl_repo')
import numpy as np

P = 128
NCORES = 8
F = 128
WSIZE = 32768          # dma_gather int16 index range
WSTARTS = (0, 22528, 45056, 67584)   # overlapping source windows
NW = 4
QUOTA = 512            # per-(block,window) edge quota for balance
GBLK = 8               # dest blocks per gather group
# Max columns (x128 idx) per dma_gather call: the SWDGE packs 16 idx per
# descriptor and a single packed gather must stay under the HW packet/ring
# entry budget (2048-idx gathers wedge the device; 1024-idx ones work).
GMAX_COLS = 8


def _prep(x, edge_index, n_pad, nblk_per_core):
    """Host-side graph prep: gcn norm, per-core dest-block/window metadata.

    Norm factorization: norm_e = dis[src]*dis[dst]. The dis[src] factor is
    folded into the stored node features (x and every h are stored scaled by
    dis), and the dis[dst] factor is applied after the aggregation matmuls.
    The scatter matrices are therefore pure 0/1 one-hots, built per dest
    block in a single broadcast is_equal, and the self-loop term reduces to
    an identity matmul of the (already dis-scaled) features.

    Edges are bucketed per (dest block, source window); each bucket is padded
    to K*128 slots where K = max over cores of ceil(count/128), so the SPMD
    instruction stream is identical on every core. Gathers are bulk
    dma_gather ops; indices are int16 window-relative source ids in the
    wrapped [16, n/16] layout replicated across the 8 Q7 cores.
    """
    N = x.shape[0]  # == n_pad (x is padded)
    loop_full = np.concatenate([edge_index[1].astype(np.int64), np.arange(N, dtype=np.int64)])
    deg = np.bincount(loop_full, minlength=N).astype(np.float32)
    dis = np.where(deg > 0, 1.0 / np.sqrt(deg), 0.0).astype(np.float32)
    row = edge_index[0].astype(np.int64)
    col = edge_index[1].astype(np.int64)

    nblk_tot = NCORES * nblk_per_core
    order = np.argsort(col, kind='stable')
    row_s = row[order]
    col_s = col[order]
    blk_of = col_s // P
    cnt_blk = np.bincount(blk_of, minlength=nblk_tot)
    starts = np.zeros(nblk_tot + 1, np.int64)
    np.cumsum(cnt_blk, out=starts[1:])

    # per (core, block): sort by src, assign to windows with quota balancing
    seg = {}
    cnts = np.zeros((NCORES, nblk_per_core, NW), np.int64)
    for c in range(NCORES):
        for j in range(nblk_per_core):
            g = c * nblk_per_core + j
            s, e = starts[g], starts[g + 1]
            so = np.argsort(row_s[s:e], kind='stable')
            srcs = row_s[s:e][so]
            dsts = col_s[s:e][so]
            ptr = 0
            for w in range(NW):
                if w < NW - 1:
                    must = int(np.searchsorted(srcs, WSTARTS[w + 1]))
                    elig = int(np.searchsorted(srcs, WSTARTS[w] + WSIZE))
                    take = max(must, min(elig, ptr + QUOTA))
                else:
                    take = len(srcs)
                cnts[c, j, w] = take - ptr
                seg[(c, j, w)] = (
                    (srcs[ptr:take] - WSTARTS[w]).astype(np.int16),
                    (dsts[ptr:take] - g * P).astype(np.float16),
                )
                ptr = take

    Khat = (-(-cnts // P)).max(axis=0)  # [nblk, NW]
    kflat = Khat.reshape(-1)
    col_start = np.zeros(nblk_per_core * NW + 1, np.int64)
    np.cumsum(kflat, out=col_start[1:])
    ncols = int(col_start[-1])

    groups = [list(range(a, min(nblk_per_core, a + GBLK)))
              for a in range(0, nblk_per_core, GBLK)]
    gw_cols = [[int(sum(Khat[j][w] for j in grp)) for w in range(NW)]
               for grp in groups]
    gw_off8 = []
    acc = 0
    for gi in range(len(groups)):
        offs = []
        for w in range(NW):
            offs.append(acc)
            acc += gw_cols[gi][w] * 8  # num_idxs/16 = cols*8 int16 per partition
        gw_off8.append(offs)
    midx_width = acc

    metas = []
    for c in range(NCORES):
        mdlc = np.full((P, ncols), -1.0, np.float16)
        midx = np.zeros((P, midx_width), np.int16)
        for gi, grp in enumerate(groups):
            for w in range(NW):
                ncgw = gw_cols[gi][w]
                if ncgw == 0:
                    continue
                arr = np.zeros(ncgw * P, np.int16)
                loc = 0
                for j in grp:
                    K = int(Khat[j][w])
                    if K == 0:
                        continue
                    src16, dlc = seg[(c, j, w)]
                    n = len(src16)
                    npad_ = K * P
                    a = np.zeros(npad_, np.int16)
                    a[:n] = src16
                    d = np.full(npad_, -1.0, np.float16)
                    d[:n] = dlc
                    c0 = int(col_start[j * NW + w])
                    # slot t of bucket -> column c0 + t//128, partition t%128
                    mdlc[:, c0:c0 + K] = d.reshape(K, P).T
                    arr[loc * P:(loc + K) * P] = a
                    loc += K
                wrapped = arr.reshape(-1, 16).T      # [16, n/16]; slot i at [i%16, i//16]
                rep = np.tile(wrapped, (8, 1))       # replicate for 8 Q7 cores
                o = gw_off8[gi][w]
                midx[:, o:o + ncgw * 8] = rep
        metas.append([midx, mdlc])

    layout = dict(ncols=ncols, Khat=Khat, col_start=col_start, groups=groups,
                  gw_cols=gw_cols, gw_off8=gw_off8, midx_width=midx_width)

    slice_rows = nblk_per_core * P
    # store features pre-scaled by dis (the dis[src] half of the edge norm)
    xs = (x * dis[:, None]).astype(np.float16)
    x_slices = [xs[c * slice_rows:(c + 1) * slice_rows] for c in range(NCORES)]
    for c in range(NCORES):
        dc = dis[c * slice_rows:(c + 1) * slice_rows].reshape(nblk_per_core, P).T
        metas[c].append(np.ascontiguousarray(dc.astype(np.float32)))  # [128, nblk]
    return metas, x_slices, xs, layout


def _build_bass(n_pad, nblk, layout, weights, biases, Wp, bp):
    """SPMD Bass program (identical on all cores; per-core data via inputs).

    Per propagation i=0..3 over scaled features v_i = h_i*dis (v0=x*dis):
      per (block-group, window): bulk dma_gather of source rows (<=1024 idx
      per call); per dest block: identity self-loop matmul plus one-hot
      scatter matmuls into gt[f,d] (f32 PSUM) using a block-wide broadcast
      is_equal one-hot, then (with the dis[dst] factor applied on the scalar
      engine):
        v_{i+1}[d,o] = relu(dis_d*(gt^T W_i) + b_i)*dis_d     (i<3)
        y'[d,2]     += gt^T Wp_{i-1}                          (i>=1)
      Epilogue: softmax(2) = sigmoid(dis_d*(y0'-y1') + bp0-bp1).
    """
    from concourse import bass, bacc, mybir
    import concourse.tile as tile

    slice_rows = nblk * P
    ncols = layout["ncols"]
    Khat = layout["Khat"]
    col_start = layout["col_start"]
    groups = layout["groups"]
    gw_cols = layout["gw_cols"]
    gw_off8 = layout["gw_off8"]
    midx_width = layout["midx_width"]

    nc = bacc.Bacc(num_devices=NCORES, num_swdge_queues=4)

    f16 = mybir.dt.float16
    f32 = mybir.dt.float32
    i16 = mybir.dt.int16
    x_in = nc.declare_dram_parameter("x_in", [slice_rows, F], f16, isOutput=False)
    x_full = nc.declare_dram_parameter("x_full", [n_pad, F], f16, isOutput=False)
    midx_in = nc.declare_dram_parameter("midx", [P, midx_width], i16, isOutput=False)
    mdlc_in = nc.declare_dram_parameter("mdlc", [P, ncols], f16, isOutput=False)
    discol_in = nc.declare_dram_parameter("discol", [P, nblk], f32, isOutput=False)
    y_out = nc.declare_dram_parameter("y_out", [slice_rows, 2], f32, isOutput=True)

    h_slice = [nc.dram_tensor(f"h_slice{i}", [slice_rows, F], f16) for i in range(3)]
    v_full = [x_full] + [nc.dram_tensor(f"v_full{i}", [n_pad, F], f16, addr_space="Shared")
                         for i in range(1, 4)]

    iota_np = np.broadcast_to(np.arange(P, dtype=np.float32), (P, P)).astype(np.float16).copy()
    iota_d = nc.inline_tensor(iota_np, name="iota_c")
    ident_d = nc.inline_tensor(np.eye(P, dtype=np.float16), name="ident")
    # fused transform weights per prop: i=0 -> W0; i=1,2 -> [W_i | Wp_{i-1}];
    # i=3 -> Wp2. One matmul then produces h' cols [0:F) and y' cols [F:F+2).
    Wp_sl = [np.ascontiguousarray(Wp[i * F:(i + 1) * F, :]).astype(np.float16)
             for i in range(3)]
    WW_np = [
        weights[0].astype(np.float16),
        np.concatenate([weights[1].astype(np.float16), Wp_sl[0]], axis=1),
        np.concatenate([weights[2].astype(np.float16), Wp_sl[1]], axis=1),
        Wp_sl[2],
    ]
    WW_d = [nc.inline_tensor(np.ascontiguousarray(w), name=f"WW{i}")
            for i, w in enumerate(WW_np)]
    B_d = [nc.inline_tensor(np.broadcast_to(b.astype(np.float32), (P, F)).copy(), name=f"B{i}")
           for i, b in enumerate(biases)]
    bp_diff = float(bp[0]) - float(bp[1])

    AF = mybir.ActivationFunctionType
    ALU = mybir.AluOpType
    rg = [list(range(NCORES))]

    with tile.TileContext(nc) as tc:
        with (
            tc.tile_pool(name="const", bufs=1) as cpool,
            tc.tile_pool(name="msg", bufs=2) as mpool,
            tc.tile_pool(name="s", bufs=6) as spool,
            tc.tile_pool(name="work", bufs=6) as wpool,
            tc.tile_pool(name="epi", bufs=1) as epool,
            tc.tile_pool(name="psA", bufs=4, space="PSUM") as psA,
            tc.tile_pool(name="psB", bufs=3, space="PSUM") as psB,
        ):
            iota_t = cpool.tile([P, P], f16)
            nc.sync.dma_start(out=iota_t[:], in_=iota_d[:, :])
            ident_t = cpool.tile([P, P], f16)
            nc.sync.dma_start(out=ident_t[:], in_=ident_d[:, :])
            WW_t, B_t = [], []
            for i in range(4):
                wt = cpool.tile([P, WW_np[i].shape[1]], f16, tag=f"ww{i}")
                nc.sync.dma_start(out=wt[:], in_=WW_d[i][:, :])
                WW_t.append(wt)
            for i in range(3):
                bt = cpool.tile([P, F], f32, tag=f"b{i}")
                nc.sync.dma_start(out=bt[:], in_=B_d[i][:, :])
                B_t.append(bt)

            midx_t = cpool.tile([P, midx_width], i16)
            mdlc_t = cpool.tile([P, ncols], f16)
            discol_t = cpool.tile([P, nblk], f32)
            nc.sync.dma_start(out=midx_t[:], in_=midx_in[:, :])
            nc.sync.dma_start(out=mdlc_t[:], in_=mdlc_in[:, :])
            nc.sync.dma_start(out=discol_t[:], in_=discol_in[:, :])

            ysb = cpool.tile([P, 2 * nblk], f32)
            nc.vector.memset(ysb[:], 0.0)
            hsl = cpool.tile([P, slice_rows], f16)
            # node-major x*dis slice for prop-0 self-loop terms
            xsl = cpool.tile([P, slice_rows], f16)
            nc.sync.dma_start(
                out=xsl[:].rearrange("d (b o) -> d b o", o=F),
                in_=x_in.rearrange("(b d) o -> d b o", d=P))

            # ---- 4 propagations ----
            # qn is a single global counter: Tile round-robins Pool-DMA insts
            # over 8 DMASW sem lanes, and each lane is locked to one SWDGE
            # queue; queue = counter % 4 keeps lane % 4 == queue always.
            qn = 0
            for i in range(4):
                src = v_full[i]
                for gi, grp in enumerate(groups):
                    # bulk-gather this group's source rows
                    mt = {}
                    for w in range(NW):
                        ncgw = gw_cols[gi][w]
                        if ncgw == 0:
                            continue
                        t = mpool.tile([P, ncgw * F], f16, tag=f"msg{w}")
                        # split into sub-gathers of <= GMAX_COLS columns to
                        # stay under the per-instruction SWDGE budget
                        for c0 in range(0, ncgw, GMAX_COLS):
                            c1 = min(c0 + GMAX_COLS, ncgw)
                            nidx = (c1 - c0) * P
                            o8 = gw_off8[gi][w] + c0 * 8
                            nc.gpsimd.dma_gather(
                                out_ap=t[:, c0 * F:c1 * F].rearrange(
                                    "p (c e) -> p c e", e=F),
                                in_ap=src[WSTARTS[w]:min(WSTARTS[w] + WSIZE, n_pad), :],
                                idxs_ap=midx_t[:, o8:o8 + (c1 - c0) * 8],
                                num_idxs=nidx,
                                num_idxs_reg=nidx,
                                elem_size=F,
                                queue_num=qn % 4,
                            )
                            qn += 1
                        mt[w] = t
                    for j in grp:
                        ktot = int(Khat[j].sum())
                        bc0 = int(col_start[j * NW])
                        gt = psA.tile([P, P], f32, tag="gt", space="PSUM")
                        # block-wide 0/1 one-hot: S[p, k, e] = (e == dlc[p, k])
                        if ktot > 0:
                            Sblk = spool.tile([P, ktot * P], f16, tag="S")
                            nc.vector.tensor_tensor(
                                out=Sblk[:].rearrange("p (k e) -> p k e", e=P),
                                in0=iota_t[:].unsqueeze(1).broadcast_to([P, ktot, P]),
                                in1=mdlc_t[:, bc0:bc0 + ktot].unsqueeze(2)
                                    .broadcast_to([P, ktot, P]),
                                op=ALU.is_equal,
                            )
                        # self-loop: stored features are already dis-scaled
                        selfsrc = xsl if i == 0 else hsl
                        nc.tensor.matmul(out=gt[:], lhsT=selfsrc[:, j * P:(j + 1) * P],
                                         rhs=ident_t[:], start=True, stop=(ktot == 0))
                        done = 0
                        for w in range(NW):
                            K = int(Khat[j][w])
                            if K == 0:
                                continue
                            loc = int(sum(Khat[j2][w] for j2 in grp if j2 < j))
                            krel = int(col_start[j * NW + w]) - bc0
                            for k in range(K):
                                lhs = mt[w][:, (loc + k) * F:(loc + k + 1) * F]
                                done += 1
                                nc.tensor.matmul(
                                    out=gt[:], lhsT=lhs,
                                    rhs=Sblk[:, (krel + k) * P:(krel + k + 1) * P],
                                    start=False, stop=(done == ktot))
                        gts = wpool.tile([P, P], f16, tag="gts")
                        nc.scalar.copy(out=gts[:], in_=gt[:])
                        nw_out = WW_np[i].shape[1]
                        hp = psB.tile([P, nw_out], f32, tag="hx", space="PSUM")
                        nc.tensor.matmul(out=hp[:], lhsT=gts[:], rhs=WW_t[i][:],
                                         start=True, stop=True)
                        if i < 3:
                            # true h = relu(dis_d*hp + b); stored scaled by dis_d
                            tmp = wpool.tile([P, P], f32, tag="tmp")
                            nc.scalar.activation(out=tmp[:], in_=hp[:, 0:F],
                                                 func=AF.Copy,
                                                 scale=discol_t[:, j:j + 1])
                            nc.vector.tensor_tensor(out=tmp[:], in0=tmp[:],
                                                    in1=B_t[i][:], op=ALU.add)
                            hb = hsl[:, j * P:(j + 1) * P]
                            nc.scalar.activation(out=hb, in_=tmp[:], func=AF.Relu,
                                                 scale=discol_t[:, j:j + 1])
                        if i >= 1:
                            ys = ysb[:, 2 * j:2 * j + 2]
                            nc.vector.tensor_tensor(out=ys, in0=ys,
                                                    in1=hp[:, nw_out - 2:nw_out],
                                                    op=ALU.add)
                    # incremental dump of this group's finished h rows
                    if i < 3:
                        b0, b1 = grp[0], grp[-1] + 1
                        nc.sync.dma_start(
                            out=h_slice[i].rearrange("(b d) o -> d b o", d=P)[:, b0:b1, :],
                            in_=hsl[:, b0 * F:b1 * F].rearrange("d (b o) -> d b o", o=F))
                if i < 3:
                    nc.gpsimd.collective_compute(
                        "AllGather", ALU.bypass, replica_groups=rg,
                        ins=[h_slice[i][:].opt()], outs=[v_full[i + 1][:].opt()],
                    )

            # ---- epilogue: softmax(2) = sigmoid(dis*(y0'-y1')+bp0-bp1) ----
            yv = ysb[:].rearrange("d (b o) -> d b o", o=2)
            dif = epool.tile([P, nblk], f32, tag="dif")
            nc.vector.tensor_tensor(out=dif[:], in0=yv[:, :, 0:1].opt(),
                                    in1=yv[:, :, 1:2].opt(), op=ALU.subtract)
            nc.vector.tensor_tensor(out=dif[:], in0=dif[:], in1=discol_t[:],
                                    op=ALU.mult)
            nc.vector.tensor_scalar_add(out=dif[:], in0=dif[:], scalar1=bp_diff)
            youtsb = epool.tile([P, 2 * nblk], f32, tag="yo")
            yov = youtsb[:].rearrange("d (b o) -> d b o", o=2)
            sig = epool.tile([P, nblk], f32, tag="sig")
            nc.scalar.activation(out=sig[:], in_=dif[:], func=AF.Sigmoid)
            nc.vector.tensor_copy(out=yov[:, :, 0:1].opt(), in_=sig[:])
            nc.vector.tensor_scalar(out=yov[:, :, 1:2].opt(), in0=sig[:],
                                    scalar1=-1.0, op0=ALU.mult,
                                    scalar2=1.0, op1=ALU.add)
            nc.sync.dma_start(
                out=y_out.rearrange("(b d) o -> d b o", d=P),
                in_=youtsb[:].rearrange("d (b o) -> d b o", o=2))

    nc.compile()
    return nc


LAST_RESULTS = None
LAST_NC = None
LAST_IN_MAPS = None


def kernel(x, edge_index, W0, b0, W1, b1, W2, b2, Wp, bp):
    global LAST_RESULTS, LAST_NC, LAST_IN_MAPS
    import os
    from concourse.bass_utils import run_bass_kernel_spmd

    x = np.asarray(x, dtype=np.float32)
    edge_index = np.asarray(edge_index)
    N = x.shape[0]
    nblk_per_core = int(np.ceil(N / (NCORES * P)))
    n_pad = NCORES * nblk_per_core * P
    slice_rows = nblk_per_core * P

    # degree-balanced relabeling: deal nodes (sorted by in-degree) round-robin
    # across all global blocks so per-block edge counts equalize, shrinking the
    # max-over-cores column padding.
    nblk_tot = NCORES * nblk_per_core
    indeg = np.bincount(edge_index[1].astype(np.int64), minlength=N)
    order_deg = np.argsort(-indeg, kind='stable')
    perm = np.empty(N, np.int64)              # old id -> new slot
    nfull = (N // nblk_tot) * nblk_tot
    nstripe = nfull // nblk_tot
    fwd = np.arange(nblk_tot)
    blk_seq = np.concatenate([fwd if s % 2 == 0 else fwd[::-1] for s in range(nstripe)])
    slot_in_blk = np.repeat(np.arange(nstripe), nblk_tot)
    newid_full = blk_seq * P + slot_in_blk
    rem = N - nfull
    newid_rem = np.arange(rem) * P + (nfull // nblk_tot)
    newid = np.concatenate([newid_full, newid_rem])
    perm[order_deg] = newid
    x_rel = np.zeros((n_pad, x.shape[1]), np.float32)
    x_rel[perm] = x
    edge_rel = perm[edge_index.astype(np.int64)]

    metas, x_slices, x_sc16, layout = _prep(x_rel, edge_rel, n_pad, nblk_per_core)

    nc = _build_bass(
        n_pad, nblk_per_core, layout,
        [np.asarray(W0), np.asarray(W1), np.asarray(W2)],
        [np.asarray(b0), np.asarray(b1), np.asarray(b2)],
        np.asarray(Wp), np.asarray(bp),
    )

    in_maps = []
    for c in range(NCORES):
        midx, mdlc, discol = metas[c]
        in_maps.append({
            "x_in": np.ascontiguousarray(x_slices[c]),
            "x_full": x_sc16,
            "midx": midx, "mdlc": mdlc, "discol": discol,
        })

    trace = bool(os.environ.get("KERNEL_TRACE"))
    res = run_bass_kernel_spmd(nc, in_maps, list(range(NCORES)), trace=trace)
    LAST_RESULTS = res
    LAST_NC = nc
    LAST_IN_MAPS = in_maps

    out = np.zeros((n_pad, 2), np.float32)
    for c in range(NCORES):
        out[c * slice_rows:(c + 1) * slice_rows] = res.results[c]["y_out"]
    return out[perm]
